# revision 8
# baseline (speedup 1.0000x reference)
import sys
sys.path.insert(0, "/opt/trn_rl_repo")
import numpy as np

NCORES = 8
N = 100000
NP = 102400          # padded node count (8 * 12800)
PC = NP // NCORES    # 12800 nodes per core
D = 256
H = 512
NS = PC // 512       # 25 supertiles of 512 dst per core
WIN = 32768
NW = (NP + WIN - 1) // WIN   # 4 source windows
BN_EPS = 1e-5


def _build_job(src, dst, n_src_tot):
    """Edge grid for one (layer, relation) job, uniform across cores.

    Returns dict with:
      idx:    [NCORES, 128, TOT//16] int16 gather indices (0-padded)
      dstloc: [NCORES, 128, M] f32 per-matmul local dst (-1 = inactive)
      segs:   list over (s) of list over (w) of (idx_off_cols, K)
      mms:    list over (s) of list of (w, c, t, col, start, stop)
    """
    core = dst // PC
    dl = dst % PC
    s_id = dl // 512
    w_id = src // WIN
    order = np.lexsort((dl, w_id, s_id, core))
    src_s, dl_s, core_s, s_s, w_s = (
        src[order], dl[order], core[order], s_id[order], w_id[order])

    # group boundaries per (core, s, w)
    key = (core_s * NS + s_s) * NW + w_s
    nkeys = NCORES * NS * NW
    starts = np.searchsorted(key, np.arange(nkeys))
    ends = np.searchsorted(key, np.arange(nkeys) + 1)

    # K per (s, w) = max over cores
    cnt = (ends - starts).reshape(NCORES, NS, NW)
    K = np.maximum(1, np.ceil(cnt / 128).astype(np.int64).max(axis=0))  # [NS, NW]

    segs, mms = [], []
    idx_blocks = [[] for _ in range(NCORES)]
    dst_blocks = [[] for _ in range(NCORES)]
    col_off = 0
    idx_off = 0
    for s in range(NS):
        seg_s, mm_s = [], []
        first_seen = {}
        last_seen = {}
        mlist = []
        for w in range(NW):
            k = int(K[s, w])
            seg_s.append((idx_off // 16, k))
            # per-core idx block + per-chunk tile spans
            spans = [set() for _ in range(k)]
            per_core_ed = []
            for r in range(NCORES):
                g = r * NS * NW + s * NW + w
                a, b = starts[g], ends[g]
                isl = np.zeros(k * 128, np.int16)
                isl[: b - a] = (src_s[a:b] - w * WIN).astype(np.int16)
                idx_blocks[r].append(isl)
                tloc = (dl_s[a:b] // 128) % 4
                per_core_ed.append((dl_s[a:b], tloc))
                for c in range(k):
                    lo, hi = c * 128, min((c + 1) * 128, b - a)
                    if hi > lo:
                        spans[c].update(np.unique(tloc[lo:hi]).tolist())
            for c in range(k):
                for t in sorted(spans[c]):
                    mlist.append((w, c, int(t), col_off))
                    # per-core dstloc column
                    for r in range(NCORES):
                        dls, tloc = per_core_ed[r]
                        colv = np.full(128, -1.0, np.float32)
                        lo, hi = c * 128, min((c + 1) * 128, len(dls))
                        if hi > lo:
                            seg = dls[lo:hi]
                            tl = tloc[lo:hi]
                            v = seg - (s * 512 + t * 128)
                            v = np.where(tl == t, v, -1).astype(np.float32)
                            colv[: hi - lo] = v
                        dst_blocks[r].append(colv)
                    col_off += 1
            idx_off += k * 128
        # coverage: every tile needs >= 1 matmul
        covered = {t for (_, _, t, _) in mlist}
        for t in range(4):
            if t not in covered:
                mlist.append((0, 0, t, col_off))
                for r in range(NCORES):
                    dst_blocks[r].append(np.full(128, -1.0, np.float32))
                col_off += 1
        for i, (w, c, t, col) in enumerate(mlist):
            if t not in first_seen:
                first_seen[t] = i
            last_seen[t] = i
        for i, (w, c, t, col) in enumerate(mlist):
            mm_s.append((w, c, t, col, i == first_seen[t], i == last_seen[t]))
        mms.append(mm_s)
        segs.append(seg_s)

    TOT = idx_off
    M = col_off
    idx = np.zeros((NCORES, 128, TOT // 16), np.int16)
    dstloc = np.zeros((NCORES, 128, M), np.float32)
    for r in range(NCORES):
        flat = np.concatenate(idx_blocks[r])
        wrapped = flat.reshape(-1, 16).T  # [16, TOT//16]
        idx[r] = np.tile(wrapped, (8, 1))
        dstloc[r] = np.stack(dst_blocks[r], axis=1)
    return {"idx": idx, "dstloc": dstloc, "segs": segs, "mms": mms,
            "TOT": TOT, "M": M}


def _wsb(Wmat):
    """[K, H] -> SBUF layout [128, (K//128)*H] fp16."""
    Kd, Hd = Wmat.shape
    out = np.zeros((128, (Kd // 128) * Hd), np.float16)
    for k in range(Kd // 128):
        out[:, k * Hd:(k + 1) * Hd] = Wmat[k * 128:(k + 1) * 128]
    return out


def _aff(g, b, m, v, extra_bias):
    A = (g / np.sqrt(v + BN_EPS)).astype(np.float32)
    B = (b - m * A + extra_bias * A).astype(np.float32)
    # per-channel -> [128, H//128]
    return A.reshape(-1, 128).T.copy(), B.reshape(-1, 128).T.copy()


def kernel(x_item, x_user, y_emb, ei_iu, ei_ui, ei_ii, params):
    import concourse.bass as bass
    import concourse.bacc as bacc
    import concourse.mybir as mybir
    from concourse import tile
    from concourse.bass_utils import run_bass_kernel_spmd

    FP16, F32, I16 = mybir.dt.float16, mybir.dt.float32, mybir.dt.int16
    AF = mybir.ActivationFunctionType
    p = params

    # ---------------- host prep ----------------
    mask = (np.asarray(y_emb) != 2).astype(np.float32)[:, None]
    xi0 = np.asarray(x_item) + np.asarray(p["emb_w"])[np.asarray(y_emb)] * mask
    xi0p = np.zeros((NP, D), np.float16); xi0p[:N] = xi0.astype(np.float16)
    xu0p = np.zeros((NP, D), np.float16); xu0p[:N] = np.asarray(x_user).astype(np.float16)

    jA = _build_job(np.asarray(ei_iu[0]), np.asarray(ei_iu[1]), NP)
    jB = _build_job(np.asarray(ei_ui[0]), np.asarray(ei_ui[1]), NP)
    jC = _build_job(np.asarray(ei_ii[0]), np.asarray(ei_ii[1]), NP)
    jD = _build_job(np.asarray(ei_ui[0]), np.asarray(ei_ui[1]), NP)
    jE = _build_job(np.asarray(ei_ii[0]), np.asarray(ei_ii[1]), NP)

    WrA = _wsb(np.asarray(p["Wr0_iu"], np.float32))
    WsA = _wsb(np.asarray(p["Ws0_iu"], np.float32))
    WrB = _wsb(np.asarray(p["Wr0_ui"], np.float32))
    WrC = _wsb(np.asarray(p["Wr0_ii"], np.float32))
    WsBC = _wsb(np.asarray(p["Ws0_ui"], np.float32) + np.asarray(p["Ws0_ii"], np.float32))
    WrD = _wsb(np.asarray(p["Wr1_ui"], np.float32))
    WrE = _wsb(np.asarray(p["Wr1_ii"], np.float32))
    WsDE = _wsb(np.asarray(p["Ws1_ui"], np.float32) + np.asarray(p["Ws1_ii"], np.float32))
    W1sb = _wsb(np.asarray(p["W1"], np.float32))
    W2sb = np.asarray(p["W2"], np.float32).astype(np.float16)  # [128, 2]

    A0u, B0u = _aff(p["bn0_user_g"], p["bn0_user_b"], p["bn0_user_m"], p["bn0_user_v"],
                    np.asarray(p["br0_iu"], np.float32))
    A0i, B0i = _aff(p["bn0_item_g"], p["bn0_item_b"], p["bn0_item_m"], p["bn0_item_v"],
                    np.asarray(p["br0_ui"], np.float32) + np.asarray(p["br0_ii"], np.float32))
    A1i, B1i = _aff(p["bn1_item_g"], p["bn1_item_b"], p["bn1_item_m"], p["bn1_item_v"],
                    np.asarray(p["br1_ui"], np.float32) + np.asarray(p["br1_ii"], np.float32))
    Af, Bf = _aff(p["bnf_g"], p["bnf_b"], p["bnf_m"], p["bnf_v"],
                  np.asarray(p["b1"], np.float32))

    iota = np.tile(np.arange(128, dtype=np.float32), (128, 1))
    ident = np.eye(128, dtype=np.float16)

    # per-core transposed local features [128, (D//128)*PC]
    def tloc(xp, r):
        blk = xp[r * PC:(r + 1) * PC]  # [PC, D]
        kd = blk.shape[1] // 128
        out = np.empty((128, kd * PC), np.float16)
        for k in range(kd):
            out[:, k * PC:(k + 1) * PC] = blk[:, k * 128:(k + 1) * 128].T
        return out

    # ---------------- bass program ----------------
    import os
    KPHASES = int(os.environ.get("KPHASES", "3"))
    nc = bacc.Bacc(None, target_bir_lowering=False)
    dp = nc.declare_dram_parameter
    xi0_d = dp("xi0", [NP, D], FP16, isOutput=False)
    xu0_d = dp("xu0", [NP, D], FP16, isOutput=False)
    xi0T_d = dp("xi0T", [128, 2 * PC], FP16, isOutput=False)
    xu0T_d = dp("xu0T", [128, 2 * PC], FP16, isOutput=False)
    meta_np = np.concatenate([iota, A0u, B0u, A0i, B0i, A1i, B1i, Af, Bf], axis=1)
    meta_d = dp("meta", list(meta_np.shape), F32, isOutput=False)
    ident_d = dp("ident", [128, 128], FP16, isOutput=False)
    wts_np = np.concatenate(
        [WrA, WsA, WrB, WrC, WsBC, WrD, WrE, WsDE, W1sb, W2sb], axis=1)
    wts_d = dp("wts", list(wts_np.shape), FP16, isOutput=False)
    jobs = {"A": jA, "B": jB, "C": jC, "D": jD, "E": jE}
    idx_d, dl_d = {}, {}
    for nm, j in jobs.items():
        idx_d[nm] = dp(f"idx{nm}", [128, j["TOT"] // 16], I16, isOutput=False)
        dl_d[nm] = dp(f"dl{nm}", [128, j["M"]], F32, isOutput=False)
    out_d = dp("out", [2, PC], F32, isOutput=True)
    xi1T_d = nc.dram_tensor("xi1T", [128, 4 * PC], FP16)

    # weight column offsets in wts
    wofs = {}
    off = 0
    for nm, arr in [("WrA", WrA), ("WsA", WsA), ("WrB", WrB), ("WrC", WrC),
                    ("WsBC", WsBC), ("WrD", WrD), ("WrE", WrE), ("WsDE", WsDE),
                    ("W1", W1sb), ("W2", W2sb)]:
        wofs[nm] = off
        off += arr.shape[1]
    mofs = {}
    off = 128
    for nm in ["A0u", "B0u", "A0i", "B0i", "A1i", "B1i", "Af", "Bf"]:
        mofs[nm] = off
        off += 4 if nm in ("A0u", "B0u", "A0i", "B0i", "A1i", "B1i") else 1

    with tile.TileContext(nc) as tc:
        with (
            tc.tile_pool(name="const", bufs=1) as cpool,
            tc.tile_pool(name="idx", bufs=3) as ipool,
            tc.tile_pool(name="dl", bufs=3) as dpool,
            tc.tile_pool(name="gath", bufs=4) as gpool,
            tc.tile_pool(name="m", bufs=2) as mpool,
            tc.tile_pool(name="aggs", bufs=3) as aspool,
            tc.tile_pool(name="aggT", bufs=2) as atpool,
            tc.tile_pool(name="xt", bufs=3) as xtpool,
            tc.tile_pool(name="xn", bufs=3) as xnpool,
            tc.tile_pool(name="row", bufs=3) as rpool,
            tc.tile_pool(name="psA", bufs=4, space="PSUM") as psA,
            tc.tile_pool(name="psD", bufs=2, space="PSUM") as psD,
            tc.tile_pool(name="psT", bufs=2, space="PSUM") as psT,
            tc.tile_pool(name="dram", bufs=1, space="DRAM") as dram,
        ):
            wts_t = cpool.tile([128, wts_np.shape[1]], FP16)
            nc.sync.dma_start(wts_t[:], wts_d[:])
            meta_t = cpool.tile([128, meta_np.shape[1]], F32)
            nc.sync.dma_start(meta_t[:], meta_d[:])
            ident_t = cpool.tile([128, 128], FP16)
            nc.sync.dma_start(ident_t[:], ident_d[:])

            xu1_sh = dram.tile([PC, H], FP16)
            xi1_sh = dram.tile([PC, H], FP16)
            xu1_full = dram.tile([NP, H], FP16)
            xi1_full = dram.tile([NP, H], FP16)

            def aff_ap(nm, h):
                base = mofs[nm]
                return meta_t[:, base + h: base + h + 1]

            def do_aggs(jn, src_d, F, s, aggT_t):
                """gather + one-hot matmuls + transpose into aggT_t
                aggT_t: [128, (F//128)*512] fp16"""
                j = jobs[jn]
                idxt = idx_d[jn]
                # load dstloc slice for this supertile
                cols = [mm[3] for mm in j["mms"][s]]
                c0, c1 = cols[0], cols[-1] + 1
                dlt = dpool.tile([128, c1 - c0], F32, tag="dl")
                nc.sync.dma_start(dlt[:], dl_d[jn][:, c0:c1])
                # gathers per window (split into <=1024-idx instructions)
                gts = []
                for w in range(NW):
                    so, K = j["segs"][s][w]
                    gt = gpool.tile([128, K * F], FP16, tag="gath")
                    wlen = min(WIN, NP - w * WIN)
                    for k0 in range(0, K, 8):
                        kk = min(8, K - k0)
                        nit = kk * 128
                        idx_sb = ipool.tile([128, nit // 16], I16, tag="idx")
                        nc.sync.dma_start(
                            idx_sb[:], idxt[:, so + k0 * 8: so + k0 * 8 + nit // 16])
                        nc.gpsimd.dma_gather(
                            out_ap=gt[:, k0 * F:(k0 + kk) * F].rearrange(
                                "p (k f) -> p k f", f=F),
                            in_ap=src_d[w * WIN: w * WIN + wlen],
                            idxs_ap=idx_sb[:],
                            num_idxs=nit,
                            num_idxs_reg=nit,
                            elem_size=F,
                        )
                    gts.append(gt)
                # one-hot matmuls into per-tile PSUM
                psum_t = {}
                for i, (w, c, t, col, st, sp) in enumerate(j["mms"][s]):
                    mt = mpool.tile([128, 128], FP16, tag="m")
                    nc.vector.tensor_tensor(
                        mt[:], meta_t[:, 0:128],
                        dlt[:, i: i + 1].broadcast_to((128, 128)),
                        mybir.AluOpType.is_equal,
                    )
                    if t not in psum_t:
                        pt_new = psA.tile([128, F], F32, tag="agg"); psum_t[t] = pt_new
                    nc.tensor.matmul(
                        psum_t[t][:],
                        mt[:, :],
                        gts[w][:, c * F:(c + 1) * F],
                        start=st, stop=sp,
                    )
                # psum -> sbuf fp16 -> transpose -> aggT slots
                for t in range(4):
                    ags = aspool.tile([128, F], FP16, tag="aggs")
                    nc.scalar.activation(ags[:], psum_t[t][:], AF.Copy)
                    for k in range(F // 128):
                        pt = psT.tile([128, 128], FP16, tag="tr")
                        nc.tensor.transpose(pt[:], ags[:, k * 128:(k + 1) * 128], ident_t[:])
                        nc.vector.tensor_copy(
                            aggT_t[:, k * 512 + t * 128: k * 512 + (t + 1) * 128], pt[:])

            def dense(s, parts, affA, affB, KF, xnT_out):
                """parts: list of (Wname, aggT_t or ('dram', tensor, kd)) contributions.
                out: xnT_out list of 4 [128,512] fp16 tiles (h-chunks)"""
                for h in range(4):
                    ps = psD.tile([128, 512], F32, tag="dense")
                    first = True
                    nmm = sum(KF // 128 for _ in parts)
                    cnt = 0
                    for (wn, src) in parts:
                        for k in range(KF // 128):
                            cnt += 1
                            if isinstance(src, tuple):
                                _, xt_t = src
                                rhs = xt_t[:, k * 512: (k + 1) * 512]
                            else:
                                rhs = src[:, k * 512:(k + 1) * 512]
                            nc.tensor.matmul(
                                ps[:], wts_t[:, wofs[wn] + k * 512 + h * 128:
                                             wofs[wn] + k * 512 + h * 128 + 128],
                                rhs, start=first, stop=(cnt == nmm))
                            first = False
                    xo = xnpool.tile([128, 512], FP16, tag=f"xn{h}")
                    nc.scalar.activation(xo[:], ps[:], AF.Relu,
                                         bias=aff_ap(affB, h), scale=aff_ap(affA, h))
                    xnT_out.append(xo)

            def store_rows(s, xnT, shard):
                """transpose h-chunk tiles back to row layout, DMA to shard."""
                for t in range(4):
                    rt = rpool.tile([128, H], FP16, tag="row")
                    for h in range(4):
                        pt = psT.tile([128, 128], FP16, tag="tr")
                        nc.tensor.transpose(pt[:], xnT[h][:, t * 128:(t + 1) * 128], ident_t[:])
                        nc.vector.tensor_copy(rt[:, h * 128:(h + 1) * 128], pt[:])
                    nc.sync.dma_start(shard[s * 512 + t * 128: s * 512 + (t + 1) * 128, :], rt[:])

            def load_xT(dram_t, kd, s):
                xt = xtpool.tile([128, kd * 512], FP16, tag="xt")
                for k in range(kd):
                    nc.sync.dma_start(
                        xt[:, k * 512:(k + 1) * 512],
                        dram_t[:, k * PC + s * 512: k * PC + (s + 1) * 512])
                return xt

            # ---- phase 1: users layer 0 ----
            for s in range(NS):
                agA = atpool.tile([128, 2 * 512], FP16, tag="atA")
                do_aggs("A", xi0_d, D, s, agA)
                xtu = load_xT(xu0T_d, 2, s)
                xn = []
                dense(s, [("WrA", agA), ("WsA", ("x", xtu))], "A0u", "B0u", D, xn)
                store_rows(s, xn, xu1_sh)
            if KPHASES >= 2:
                nc.gpsimd.collective_compute(
                    "AllGather", mybir.AluOpType.bypass,
                    replica_groups=[list(range(NCORES))],
                    ins=[xu1_sh.opt()], outs=[xu1_full.opt()])

            # ---- phase 2: items layer 0 ----
            for s in range(NS if KPHASES >= 2 else 0):
                agB = atpool.tile([128, 2 * 512], FP16, tag="atB")
                do_aggs("B", xu0_d, D, s, agB)
                agC = atpool.tile([128, 2 * 512], FP16, tag="atC")
                do_aggs("C", xi0_d, D, s, agC)
                xti = load_xT(xi0T_d, 2, s)
                xn = []
                dense(s, [("WrB", agB), ("WrC", agC), ("WsBC", ("x", xti))],
                      "A0i", "B0i", D, xn)
                store_rows(s, xn, xi1_sh)
                # also store transposed for layer1 root + JK
                for h in range(4):
                    nc.sync.dma_start(
                        xi1T_d[:, h * PC + s * 512: h * PC + (s + 1) * 512], xn[h][:])
            if KPHASES >= 3:
                nc.gpsimd.collective_compute(
                    "AllGather", mybir.AluOpType.bypass,
                    replica_groups=[list(range(NCORES))],
                    ins=[xi1_sh.opt()], outs=[xi1_full.opt()])

            # ---- phase 3: items layer 1 + JK + MLP ----
            for s in range(NS if KPHASES >= 3 else 0):
                agD = atpool.tile([128, 4 * 512], FP16, tag="atD")
                do_aggs("D", xu1_full, H, s, agD)
                agE = atpool.tile([128, 4 * 512], FP16, tag="atE")
                do_aggs("E", xi1_full, H, s, agE)
                xti1 = load_xT(xi1T_d, 4, s)
                xn2 = []
                dense(s, [("WrD", agD), ("WrE", agE), ("WsDE", ("x", xti1))],
                      "A1i", "B1i", H, xn2)
                # JK: h = relu(bnf(cat(xi0,xi1,xi2) @ W1))
                xti0 = load_xT(xi0T_d, 2, s)
                ps = psD.tile([128, 512], F32, tag="dense")
                kidx = 0
                for k in range(2):
                    nc.tensor.matmul(ps[:], wts_t[:, wofs["W1"] + kidx * 128: wofs["W1"] + kidx * 128 + 128],
                                     xti0[:, k * 512:(k + 1) * 512], start=(kidx == 0), stop=False)
                    kidx += 1
                for k in range(4):
                    nc.tensor.matmul(ps[:], wts_t[:, wofs["W1"] + kidx * 128: wofs["W1"] + kidx * 128 + 128],
                                     xti1[:, k * 512:(k + 1) * 512], start=False, stop=False)
                    kidx += 1
                for k in range(4):
                    nc.tensor.matmul(ps[:], wts_t[:, wofs["W1"] + kidx * 128: wofs["W1"] + kidx * 128 + 128],
                                     xn2[k][:], start=False, stop=(kidx == 9))
                    kidx += 1
                hT = xnpool.tile([128, 512], FP16, tag="hT")
                nc.scalar.activation(hT[:], ps[:], AF.Relu,
                                     bias=aff_ap("Bf", 0), scale=aff_ap("Af", 0))
                ps2 = psT.tile([2, 512], F32, tag="tr")
                nc.tensor.matmul(ps2[:], wts_t[:, wofs["W2"]: wofs["W2"] + 2], hT[:],
                                 start=True, stop=True)
                ot = rpool.tile([2, 512], F32, tag="ot")
                nc.scalar.activation(ot[:], ps2[:], AF.Copy)
                nc.sync.dma_start(out_d[:, s * 512:(s + 1) * 512], ot[:])

            if KPHASES < 3:
                for s in range(NS):
                    zt = rpool.tile([2, 512], F32, tag="ot")
                    nc.vector.memset(zt[:], 0.0)
                    nc.sync.dma_start(out_d[:, s * 512:(s + 1) * 512], zt[:])

    nc.finalize()

    in_maps = []
    for r in range(NCORES):
        m = {
            "xi0": xi0p, "xu0": xu0p,
            "xi0T": tloc(xi0p, r), "xu0T": tloc(xu0p, r),
            "meta": meta_np, "ident": ident, "wts": wts_np,
        }
        for nm, j in jobs.items():
            m[f"idx{nm}"] = j["idx"][r]
            m[f"dl{nm}"] = j["dstloc"][r]
        in_maps.append(m)

    res = run_bass_kernel_spmd(nc, in_maps, list(range(NCORES)),
                               trace=bool(int(os.environ.get("KTRACE", "0"))))
    if res.exec_time_ns is not None:
        print(f"HW exec time: {res.exec_time_ns} ns")
    outs = [res.results[r]["out"] for r in range(NCORES)]
    full = np.concatenate(outs, axis=1).T[:N]  # [N, 2]
    return (full + np.asarray(p["b2"], np.float32)[None, :]).astype(np.float32)
